# revision 30
# baseline (speedup 1.0000x reference)
"""Masked L1 loss (anomaly VQ loss) on 8 Trainium2 NeuronCores.

reference math:
    num = sum(|pred - vq[c]| * (1 - mask))   over (N,V,C,T,H,W)
    den = sum(1 - mask) * V*C*T              (mask broadcast over V,C,T)
    out = num / den

Sharding: data-parallel over the batch axis N=8 -> one batch element per core.

Per-core pipeline (fp8 end-to-end), built around two identities:
    |d| = d - 2*min(d, 0),      d = x - vq[c]
    sum(acc*(1-m)) = sum(acc) - sum(acc*m)
so most elements need only ONE elementwise op, and the mask never sits on
the critical path (no wm tile at all).

  pred: host-cast to float8_e4m3 (HBM traffic /4; kernel is HBM-read bound)
  and laid out c-pair-major [12, 128, 6144] so each DMA is one contiguous
  6KB-per-partition transfer.  p = t*16+(h>>3), f = (h&7)*128+w, s = c2*3+v.

  Lanes, assigned per c-pair:
   d1 (DVE): mt = min(x - vq_c, 0) fp8, one fused op per c-group (1814ns;
      a classic 2-op abs costs 2779ns).  PE accumulates BOTH the raw slabs
      (stationary [I|I], DoubleRow fp8: two slabs per pass) and the min
      slabs (stationary [-2I|-2I]); PSUM gets sum(|d| + vq_c).  The
      vq_c*sum(1-m) surplus is subtracted on the host.
   act (ACT): at = Abs(-x + vq_c) fused, fp8 out; PE accumulates DR pairs.

  Schedule notes (all from HW traces):
   - consts are padded to >=512B/partition (smaller descriptors hit the
     SDMA read-modify-write path and poison the stream head).
   - mask DMAs ride the sync ring mid-stream: as SWDGE they throttled the
     early DMA ramp (256 tiny packets during the first pred transfers).
   - a dummy 1-element activation right after the consts hoists the 1.3us
     ACT_TABLE_LOAD (and its inherited data wait) off the first real abs.
   - DVE's first pair (cp1) is DMA'd before ACT's (cp0): ACT can't start
     before its table load anyway.
   - the last c-pair is d1 with per-slab DMAs and min ops; its matmuls run
     half0-first so the half-0 epilogue overlaps the half-1 matmuls.

  epilogue: per PSUM half, DVE scalar_tensor_tensor gives sum(acc*m) with
  accum_out; ACT (idle by then) accumulates sum(acc) per half and sum(m).
  Host: S_wm = 131072 - sum(m); num = sum(acc) - sum(acc*m)
        - 3*S_wm*sum(vq_c over d1 pairs);  out = num / (V*C*S_wm).
"""

import os
import sys

for _p in ("/opt/trn_rl_repo", "/root/.axon_site/_ro/trn_rl_repo"):
    if os.path.isdir(_p) and _p not in sys.path:
        sys.path.insert(0, _p)

import numpy as np

import concourse.bacc as bacc
import concourse.mybir as mybir
import concourse.tile as tile
from concourse.bass_utils import run_bass_kernel_spmd

N_CORES = 8
V, C, T, H, W = 3, 24, 8, 128, 128
P = 128
FD = T * W          # 1024 free elements per slab
S = 2 * V           # 6 slabs per c-pair
CP = C // 2         # 12 c-pairs
HALF = FD // 2      # 512 = one PSUM bank of fp32

F32 = mybir.dt.float32
BF16 = mybir.dt.bfloat16
FP8 = mybir.dt.float8e4

ALU = mybir.AluOpType
ACTF = mybir.ActivationFunctionType
DR = mybir.MatmulPerfMode.DoubleRow

# lane per c-pair: "d1" = DVE min-trick, "act" = ACT fused-abs.
# 7 d1 / 5 act balances DVE(1814/grp) vs ACT(2838/grp).  The surplus d1
# pairs sit mid-stream (positions 3,4 and 8,9) where the DMA stream runs
# ahead of the lanes; the tail alternates act,d1 so neither engine ends
# with a serial run.
LANES = ("act", "d1", "act", "d1", "act", "d1", "act", "d1", "d1", "d1", "act", "d1")


def build_nc(lanes=LANES):
    nc = bacc.Bacc("TRN2", target_bir_lowering=False, debug=False)

    pred = nc.declare_dram_parameter("pred", [CP, P, S * FD], FP8, isOutput=False)
    # mask pre-replicated to all 128 partitions on the host: a [16,1024]
    # slice-DMA writes through only 2 of 16 SBUF ports and, the HWDGE ring
    # being FIFO, stalls the pred stream behind it
    m_d = nc.declare_dram_parameter("m_host", [P, FD], FP8, isOutput=False)
    # all consts packed in one transfer (3 separate small DMAs each paid
    # ~1.5-2us of serial completion latency at the head of the stream):
    # bytes 0:512 vqb f32[128], 512:768 [I|I] fp8, 768:1024 [-2I|-2I] fp8
    con_d = nc.declare_dram_parameter("consts_host", [P, 1024], mybir.dt.uint8, isOutput=False)
    out = nc.declare_dram_parameter("out", [P, 8], F32, isOutput=True)

    with tile.TileContext(nc) as tc:
        with (
            tc.tile_pool(name="const", bufs=1) as constp,
            tc.tile_pool(name="predp", bufs=CP) as predp,
            tc.tile_pool(name="absp", bufs=5) as absp,
            tc.tile_pool(name="psum", bufs=1, space="PSUM") as psump,
            tc.tile_pool(name="fin", bufs=1) as finp,
        ):
            con = constp.tile([P, 1024], mybir.dt.uint8)
            m8 = finp.tile([P, FD], FP8)
            nc.sync.dma_start(con[:, :], con_d[:, :])
            vqb = con[:, 0:512].bitcast(F32)
            wpos = con[:, 512:768].bitcast(FP8).rearrange("p (two m) -> p two m", two=2)
            wneg = con[:, 768:1024].bitcast(FP8).rearrange("p (two m) -> p two m", two=2)

            # hoists ACT_TABLE_LOAD off the first real abs
            dummy = constp.tile([P, 1], F32)
            nc.scalar.activation(
                dummy[:, 0:1], vqb[:, 0:1], ACTF.Abs,
                bias=vqb[:, 0:1], scale=-1.0,
            )

            # --- queue the pred DMAs up front.  cp0 (act) and cp1 (d1)
            # interleave at group granularity so BOTH lanes start by ~10us
            pts = [None] * CP
            pt0 = predp.tile([P, S, FD], FP8, tag="pt", name="pt0")
            pt1 = predp.tile([P, S, FD], FP8, tag="pt", name="pt1")
            pts[0], pts[1] = pt0, pt1
            for g in range(2):
                for cp in (0, 1):
                    nc.sync.dma_start(
                        pts[cp][:, 3 * g : 3 * (g + 1), :],
                        pred[cp][:, 3 * g * FD : 3 * (g + 1) * FD],
                    )
            for cp in range(2, CP):
                pt = predp.tile([P, S, FD], FP8, tag="pt")
                if cp == CP - 1:
                    # last pair per slab-pair: shortest possible tail
                    for q in range(3):
                        nc.sync.dma_start(
                            pt[:, 2 * q : 2 * q + 2, :],
                            pred[cp][:, 2 * q * FD : (2 * q + 2) * FD],
                        )
                else:
                    nc.sync.dma_start(pt[:, :, :], pred[cp])
                pts[cp] = pt
                if cp == 5:
                    # single full-width mask DMA mid-stream
                    nc.sync.dma_start(m8[:, :], m_d[:, :])

            accA = psump.tile([P, HALF], F32)
            accB = psump.tile([P, HALF], F32)
            accs = (accA, accB)
            r4 = finp.tile([P, 8], F32)

            # matmul start/stop bookkeeping per PSUM half
            n_mm = [0, 0]
            total_mm = [0, 0]
            for cp in range(CP):
                total_mm[0] += 6 if lanes[cp] == "d1" else 3
                total_mm[1] += 6 if lanes[cp] == "d1" else 3

            def mm(h, stat, rhs_t, q):
                first = n_mm[h] == 0
                n_mm[h] += 1
                last = n_mm[h] == total_mm[h]
                nc.tensor.matmul(
                    accs[h][:, :],
                    stat[:, :, :],
                    rhs_t[:, 2 * q : 2 * q + 2, h * HALF : (h + 1) * HALF],
                    start=first, stop=last, perf_mode=DR,
                )

            def epilogue_half(h):
                # sum(acc_h * m) on DVE; sum(acc_h) on ACT (idle by now)
                junk = finp.tile([P, HALF], BF16)
                nc.vector.scalar_tensor_tensor(
                    junk[:, :], accs[h][:, :], 1.0,
                    m8[:, h * HALF : (h + 1) * HALF],
                    op0=ALU.mult, op1=ALU.mult, accum_out=r4[:, h : h + 1],
                )
                junk2 = finp.tile([P, HALF], F32)
                nc.scalar.activation(
                    junk2[:, :], accs[h][:, :],
                    ACTF.Identity, accum_out=r4[:, 3 + h : 4 + h],
                )

            # --- main loop over c-pairs -----------------------------------
            # PE FIFO discipline: raw matmuls (DMA-gated) are emitted at
            # their own cp; lane-produced (min/abs) matmuls lag their data
            # by 2-6us, so they are emitted one cp LATER to keep the PE
            # FIFO in expected-readiness order (else it head-of-line
            # blocks and drains in a serial bunch at the end).
            pending = []
            stash = []
            last_act = max(i for i in range(CP) if lanes[i] == "act")

            def flush_pending():
                for stat_p, t_p, is_last_act in pending:
                    if is_last_act:
                        # the last act abs finishes ~41.5us; its matmuls go
                        # at the very end of the PE FIFO (readiness order)
                        stash.append((stat_p, t_p))
                        continue
                    for h in (0, 1):
                        for q in range(3):
                            mm(h, stat_p, t_p, q)
                pending.clear()

            for cp in range(CP):
                if cp == 8:
                    nc.vector.tensor_reduce(
                        r4[:, 2:3], m8[:, :], axis=mybir.AxisListType.X, op=ALU.add
                    )
                pt = pts[cp]
                last_cp = cp == CP - 1
                if lanes[cp] == "d1":
                    # raw slabs accumulate straight off the DMA
                    for h in (0, 1):
                        for q in range(3):
                            mm(h, wpos, pt, q)
                flush_pending()
                if lanes[cp] == "act":
                    at = absp.tile([P, S, FD], FP8, tag="at")
                    for c2 in (0, 1):
                        c = 2 * cp + c2
                        nc.scalar.activation(
                            at[:, 3 * c2 : 3 * (c2 + 1), :],
                            pt[:, 3 * c2 : 3 * (c2 + 1), :],
                            ACTF.Abs, bias=vqb[:, c : c + 1], scale=-1.0,
                        )
                    pending.append((wpos, at, cp == last_act))
                else:
                    mt = absp.tile([P, S, FD], FP8, tag="at")
                    if last_cp:
                        # slab-granular min ops chase the per-slab-pair DMAs
                        for s in range(S):
                            c = 2 * cp + (s // 3)
                            nc.vector.tensor_scalar(
                                mt[:, s, :], pt[:, s, :], vqb[:, c : c + 1], 0.0,
                                op0=ALU.subtract, op1=ALU.min,
                            )
                    else:
                        for c2 in (0, 1):
                            c = 2 * cp + c2
                            nc.vector.tensor_scalar(
                                mt[:, 3 * c2 : 3 * (c2 + 1), :],
                                pt[:, 3 * c2 : 3 * (c2 + 1), :],
                                vqb[:, c : c + 1], 0.0,
                                op0=ALU.subtract, op1=ALU.min,
                            )
                    pending.append((wneg, mt, False))

            # tail: everything left, the last cp's min matmuls at the end;
            # half 0 first so its epilogue overlaps the half-1 matmuls
            _, mt_last, _ = pending.pop()
            flush_pending()
            for h in (0, 1):
                for q in range(3):
                    mm(h, wneg, mt_last, q)
                for stat_s, t_s in stash:
                    for q in range(3):
                        mm(h, stat_s, t_s, q)
                epilogue_half(h)

            nc.sync.dma_start(out[:, :], r4[:, :])

    nc.compile()
    return nc


_NC_CACHE = None


def _get_nc():
    global _NC_CACHE
    if _NC_CACHE is None:
        _NC_CACHE = build_nc()
    return _NC_CACHE


def make_in_maps(pred, mask_extreme, vq_0):
    import ml_dtypes

    fp8 = ml_dtypes.float8_e4m3fn
    pred8 = np.asarray(pred).astype(fp8)
    # (N,V,C,T,H,W) -> per core [cp, p, c2, v, f] contiguous
    x = pred8.reshape(N_CORES, V, C, P, FD)
    x = x.transpose(0, 2, 3, 1, 4)                  # (N, C, P, V, FD)
    x = x.reshape(N_CORES, CP, 2, P, V, FD)
    x = np.ascontiguousarray(x.transpose(0, 1, 3, 2, 4, 5))  # (N, CP, P, 2, V, FD)
    x = x.reshape(N_CORES, CP, P, S * FD)

    m_host = np.asarray(mask_extreme, dtype=np.int32).astype(fp8)
    m_host = m_host.reshape(N_CORES, 16, FD)
    m_host = np.ascontiguousarray(np.tile(m_host, (1, T, 1)))  # (N, 128, FD)

    vq_0 = np.ascontiguousarray(vq_0, dtype=np.float32)
    vqb = np.zeros((P, P), dtype=np.float32)
    vqb[:, :C] = np.tile(vq_0, (P, 1))
    eye = np.eye(P, dtype=np.float32)
    wpos = np.concatenate([eye, eye], axis=1).astype(fp8)
    wneg = np.concatenate([-2 * eye, -2 * eye], axis=1).astype(fp8)
    con = np.zeros((P, 1024), dtype=np.uint8)
    con[:, 0:512] = vqb.view(np.uint8)
    con[:, 512:768] = wpos.view(np.uint8)
    con[:, 768:1024] = wneg.view(np.uint8)
    con = np.ascontiguousarray(con)

    in_maps = []
    for i in range(N_CORES):
        in_maps.append(
            {
                "pred": x[i],
                "m_host": m_host[i],
                "consts_host": con,
            }
        )
    return in_maps


# host-side vq correction: d1 lanes accumulate sum(x) - 2*sum(min) whose
# mask-weighted sum exceeds sum(wm*|d|) by vq_c * S_wm per slab (3 slabs
# per c-group)
D1_CS = [c for cp in range(CP) if LANES[cp] == "d1" for c in (2 * cp, 2 * cp + 1)]


def combine(results, vq_0):
    vq64 = np.asarray(vq_0, dtype=np.float64).reshape(-1)
    vq_d1 = float(vq64[D1_CS].sum())
    num = 0.0
    wsum = 0.0
    for r in results:
        o = np.asarray(r["out"], dtype=np.float64)  # [128, 8] per-partition partials
        s_m = o[:, 2].sum()                  # sum of mask over [128,1024]
        s_wm = float(P * FD) - s_m           # sum of (1-mask), T-replicated
        acc_sum = o[:, 3].sum() + o[:, 4].sum()
        acc_m = o[:, 0].sum() + o[:, 1].sum()
        num += (acc_sum - acc_m) - 3.0 * s_wm * vq_d1
        wsum += s_wm
    den = wsum * float(V * C)  # wsum already counts each mask element T times
    return np.array(num / den, dtype=np.float32)


def kernel(pred, mask_extreme, vq_0):
    nc = _get_nc()
    in_maps = make_in_maps(pred, mask_extreme, vq_0)
    res = run_bass_kernel_spmd(nc, in_maps, core_ids=list(range(N_CORES)))
    return combine(res.results, vq_0)


if __name__ == "__main__":
    rng = np.random.default_rng(0)
    pred = rng.standard_normal((8, V, C, T, H, W), dtype=np.float32)
    mask = rng.integers(0, 2, size=(8, H, W)).astype(np.int32)
    vq = rng.standard_normal((1, C), dtype=np.float32)
    got = kernel(pred=pred, mask_extreme=mask, vq_0=vq)
    m = mask.astype(np.float64)[:, None, None, None, :, :]
    w = 1.0 - m
    p64 = pred.astype(np.float64)
    numr = np.abs(p64 - vq.astype(np.float64)[0][None, None, :, None, None, None]) * w
    exp = numr.sum() / (w.sum() * V * C * T)
    print("kernel:", got, "expected:", exp, "rel:", abs(got - exp) / abs(exp))


# revision 31
# speedup vs baseline: 1.0153x; 1.0153x over previous
"""Masked L1 loss (anomaly VQ loss) on 8 Trainium2 NeuronCores.

reference math:
    num = sum(|pred - vq[c]| * (1 - mask))   over (N,V,C,T,H,W)
    den = sum(1 - mask) * V*C*T              (mask broadcast over V,C,T)
    out = num / den

Sharding: data-parallel over the batch axis N=8 -> one batch element per core.

Per-core pipeline (fp8 end-to-end), built around two identities:
    |d| = d - 2*min(d, 0),      d = x - vq[c]
    sum(acc*(1-m)) = sum(acc) - sum(acc*m)
so most elements need only ONE elementwise op, and the mask never sits on
the critical path (no wm tile at all).

  pred: host-cast to float8_e4m3 (HBM traffic /4; kernel is HBM-read bound)
  and laid out c-pair-major [12, 128, 6144] so each DMA is one contiguous
  6KB-per-partition transfer.  p = t*16+(h>>3), f = (h&7)*128+w, s = c2*3+v.

  Lanes, assigned per c-pair:
   d1 (DVE): mt = min(x - vq_c, 0) fp8, one fused op per c-group (1814ns;
      a classic 2-op abs costs 2779ns).  PE accumulates BOTH the raw slabs
      (stationary [I|I], DoubleRow fp8: two slabs per pass) and the min
      slabs (stationary [-2I|-2I]); PSUM gets sum(|d| + vq_c).  The
      vq_c*sum(1-m) surplus is subtracted on the host.
   act (ACT): at = Abs(-x + vq_c) fused, fp8 out; PE accumulates DR pairs.

  Schedule notes (all from HW traces):
   - consts are padded to >=512B/partition (smaller descriptors hit the
     SDMA read-modify-write path and poison the stream head).
   - mask DMAs ride the sync ring mid-stream: as SWDGE they throttled the
     early DMA ramp (256 tiny packets during the first pred transfers).
   - a dummy 1-element activation right after the consts hoists the 1.3us
     ACT_TABLE_LOAD (and its inherited data wait) off the first real abs.
   - DVE's first pair (cp1) is DMA'd before ACT's (cp0): ACT can't start
     before its table load anyway.
   - the last c-pair is d1 with per-slab DMAs and min ops; its matmuls run
     half0-first so the half-0 epilogue overlaps the half-1 matmuls.

  epilogue: per PSUM half, DVE scalar_tensor_tensor gives sum(acc*m) with
  accum_out; ACT (idle by then) accumulates sum(acc) per half and sum(m).
  Host: S_wm = 131072 - sum(m); num = sum(acc) - sum(acc*m)
        - 3*S_wm*sum(vq_c over d1 pairs);  out = num / (V*C*S_wm).
"""

import os
import sys

for _p in ("/opt/trn_rl_repo", "/root/.axon_site/_ro/trn_rl_repo"):
    if os.path.isdir(_p) and _p not in sys.path:
        sys.path.insert(0, _p)

import numpy as np

import concourse.bacc as bacc
import concourse.mybir as mybir
import concourse.tile as tile
from concourse.bass_utils import run_bass_kernel_spmd

N_CORES = 8
V, C, T, H, W = 3, 24, 8, 128, 128
P = 128
FD = T * W          # 1024 free elements per slab
S = 2 * V           # 6 slabs per c-pair
CP = C // 2         # 12 c-pairs
HALF = FD // 2      # 512 = one PSUM bank of fp32

F32 = mybir.dt.float32
BF16 = mybir.dt.bfloat16
FP8 = mybir.dt.float8e4

ALU = mybir.AluOpType
ACTF = mybir.ActivationFunctionType
DR = mybir.MatmulPerfMode.DoubleRow

# lane per c-pair: "d1" = DVE min-trick, "act" = ACT fused-abs.
# 7 d1 / 5 act balances DVE(1814/grp) vs ACT(2838/grp).  The surplus d1
# pairs sit mid-stream (positions 3,4 and 8,9) where the DMA stream runs
# ahead of the lanes; the tail alternates act,d1 so neither engine ends
# with a serial run.
LANES = ("act", "d1", "act", "d1", "act", "d1", "d1", "d1", "act", "d1", "act", "d1")


def build_nc(lanes=LANES):
    nc = bacc.Bacc("TRN2", target_bir_lowering=False, debug=False)

    pred = nc.declare_dram_parameter("pred", [CP, P, S * FD], FP8, isOutput=False)
    # mask pre-replicated to all 128 partitions on the host: a [16,1024]
    # slice-DMA writes through only 2 of 16 SBUF ports and, the HWDGE ring
    # being FIFO, stalls the pred stream behind it
    m_d = nc.declare_dram_parameter("m_host", [P, FD], FP8, isOutput=False)
    # all consts packed in one transfer (3 separate small DMAs each paid
    # ~1.5-2us of serial completion latency at the head of the stream):
    # bytes 0:512 vqb f32[128], 512:768 [I|I] fp8, 768:1024 [-2I|-2I] fp8
    con_d = nc.declare_dram_parameter("consts_host", [P, 1024], mybir.dt.uint8, isOutput=False)
    out = nc.declare_dram_parameter("out", [P, 8], F32, isOutput=True)

    with tile.TileContext(nc) as tc:
        with (
            tc.tile_pool(name="const", bufs=1) as constp,
            tc.tile_pool(name="predp", bufs=CP) as predp,
            tc.tile_pool(name="absp", bufs=5) as absp,
            tc.tile_pool(name="psum", bufs=1, space="PSUM") as psump,
            tc.tile_pool(name="fin", bufs=1) as finp,
        ):
            con = constp.tile([P, 1024], mybir.dt.uint8)
            m8 = finp.tile([P, FD], FP8)
            nc.sync.dma_start(con[:, :], con_d[:, :])
            vqb = con[:, 0:512].bitcast(F32)
            wpos = con[:, 512:768].bitcast(FP8).rearrange("p (two m) -> p two m", two=2)
            wneg = con[:, 768:1024].bitcast(FP8).rearrange("p (two m) -> p two m", two=2)

            # hoists ACT_TABLE_LOAD off the first real abs
            dummy = constp.tile([P, 1], F32)
            nc.scalar.activation(
                dummy[:, 0:1], vqb[:, 0:1], ACTF.Abs,
                bias=vqb[:, 0:1], scale=-1.0,
            )

            # --- queue the pred DMAs up front.  cp0 (act) and cp1 (d1)
            # interleave at group granularity so BOTH lanes start by ~10us
            pts = [None] * CP
            pt0 = predp.tile([P, S, FD], FP8, tag="pt", name="pt0")
            pt1 = predp.tile([P, S, FD], FP8, tag="pt", name="pt1")
            pts[0], pts[1] = pt0, pt1
            for g in range(2):
                for cp in (0, 1):
                    nc.sync.dma_start(
                        pts[cp][:, 3 * g : 3 * (g + 1), :],
                        pred[cp][:, 3 * g * FD : 3 * (g + 1) * FD],
                    )
            for cp in range(2, CP):
                pt = predp.tile([P, S, FD], FP8, tag="pt")
                if cp == CP - 1:
                    # last pair per slab-pair: shortest possible tail
                    for q in range(3):
                        nc.sync.dma_start(
                            pt[:, 2 * q : 2 * q + 2, :],
                            pred[cp][:, 2 * q * FD : (2 * q + 2) * FD],
                        )
                else:
                    nc.sync.dma_start(pt[:, :, :], pred[cp])
                pts[cp] = pt
                if cp == 5:
                    # single full-width mask DMA mid-stream
                    nc.sync.dma_start(m8[:, :], m_d[:, :])

            accA = psump.tile([P, HALF], F32)
            accB = psump.tile([P, HALF], F32)
            accs = (accA, accB)
            r4 = finp.tile([P, 8], F32)

            # matmul start/stop bookkeeping per PSUM half
            n_mm = [0, 0]
            total_mm = [0, 0]
            for cp in range(CP):
                total_mm[0] += 6 if lanes[cp] == "d1" else 3
                total_mm[1] += 6 if lanes[cp] == "d1" else 3

            def mm(h, stat, rhs_t, q):
                first = n_mm[h] == 0
                n_mm[h] += 1
                last = n_mm[h] == total_mm[h]
                nc.tensor.matmul(
                    accs[h][:, :],
                    stat[:, :, :],
                    rhs_t[:, 2 * q : 2 * q + 2, h * HALF : (h + 1) * HALF],
                    start=first, stop=last, perf_mode=DR,
                )

            def epilogue_half(h):
                # sum(acc_h * m) on DVE; sum(acc_h) on ACT (idle by now)
                junk = finp.tile([P, HALF], BF16)
                nc.vector.scalar_tensor_tensor(
                    junk[:, :], accs[h][:, :], 1.0,
                    m8[:, h * HALF : (h + 1) * HALF],
                    op0=ALU.mult, op1=ALU.mult, accum_out=r4[:, h : h + 1],
                )
                junk2 = finp.tile([P, HALF], F32)
                nc.scalar.activation(
                    junk2[:, :], accs[h][:, :],
                    ACTF.Identity, accum_out=r4[:, 3 + h : 4 + h],
                )

            # --- main loop over c-pairs -----------------------------------
            # PE FIFO discipline: raw matmuls (DMA-gated) are emitted at
            # their own cp; lane-produced (min/abs) matmuls lag their data
            # by 2-6us, so they are emitted one cp LATER to keep the PE
            # FIFO in expected-readiness order (else it head-of-line
            # blocks and drains in a serial bunch at the end).
            pending = []
            stash = []
            last_act = max(i for i in range(CP) if lanes[i] == "act")

            def flush_pending():
                for stat_p, t_p, is_last_act in pending:
                    if is_last_act:
                        # the last act abs finishes ~41.5us; its matmuls go
                        # at the very end of the PE FIFO (readiness order)
                        stash.append((stat_p, t_p))
                        continue
                    for h in (0, 1):
                        for q in range(3):
                            mm(h, stat_p, t_p, q)
                pending.clear()

            for cp in range(CP):
                if cp == 8:
                    nc.vector.tensor_reduce(
                        r4[:, 2:3], m8[:, :], axis=mybir.AxisListType.X, op=ALU.add
                    )
                pt = pts[cp]
                last_cp = cp == CP - 1
                if lanes[cp] == "d1":
                    # raw slabs accumulate straight off the DMA
                    for h in (0, 1):
                        for q in range(3):
                            mm(h, wpos, pt, q)
                flush_pending()
                if lanes[cp] == "act":
                    at = absp.tile([P, S, FD], FP8, tag="at")
                    for c2 in (0, 1):
                        c = 2 * cp + c2
                        nc.scalar.activation(
                            at[:, 3 * c2 : 3 * (c2 + 1), :],
                            pt[:, 3 * c2 : 3 * (c2 + 1), :],
                            ACTF.Abs, bias=vqb[:, c : c + 1], scale=-1.0,
                        )
                    pending.append((wpos, at, cp == last_act))
                else:
                    mt = absp.tile([P, S, FD], FP8, tag="at")
                    if last_cp:
                        # slab-granular min ops chase the per-slab-pair DMAs
                        for s in range(S):
                            c = 2 * cp + (s // 3)
                            nc.vector.tensor_scalar(
                                mt[:, s, :], pt[:, s, :], vqb[:, c : c + 1], 0.0,
                                op0=ALU.subtract, op1=ALU.min,
                            )
                    else:
                        for c2 in (0, 1):
                            c = 2 * cp + c2
                            nc.vector.tensor_scalar(
                                mt[:, 3 * c2 : 3 * (c2 + 1), :],
                                pt[:, 3 * c2 : 3 * (c2 + 1), :],
                                vqb[:, c : c + 1], 0.0,
                                op0=ALU.subtract, op1=ALU.min,
                            )
                    pending.append((wneg, mt, False))

            # tail: everything left, the last cp's min matmuls at the end;
            # half 0 first so its epilogue overlaps the half-1 matmuls
            _, mt_last, _ = pending.pop()
            flush_pending()
            for h in (0, 1):
                for q in range(3):
                    mm(h, wneg, mt_last, q)
                for stat_s, t_s in stash:
                    for q in range(3):
                        mm(h, stat_s, t_s, q)
                epilogue_half(h)

            nc.sync.dma_start(out[:, :], r4[:, :])

    nc.compile()
    return nc


_NC_CACHE = None


def _get_nc():
    global _NC_CACHE
    if _NC_CACHE is None:
        _NC_CACHE = build_nc()
    return _NC_CACHE


def make_in_maps(pred, mask_extreme, vq_0):
    import ml_dtypes

    fp8 = ml_dtypes.float8_e4m3fn
    pred8 = np.asarray(pred).astype(fp8)
    # (N,V,C,T,H,W) -> per core [cp, p, c2, v, f] contiguous
    x = pred8.reshape(N_CORES, V, C, P, FD)
    x = x.transpose(0, 2, 3, 1, 4)                  # (N, C, P, V, FD)
    x = x.reshape(N_CORES, CP, 2, P, V, FD)
    x = np.ascontiguousarray(x.transpose(0, 1, 3, 2, 4, 5))  # (N, CP, P, 2, V, FD)
    x = x.reshape(N_CORES, CP, P, S * FD)

    m_host = np.asarray(mask_extreme, dtype=np.int32).astype(fp8)
    m_host = m_host.reshape(N_CORES, 16, FD)
    m_host = np.ascontiguousarray(np.tile(m_host, (1, T, 1)))  # (N, 128, FD)

    vq_0 = np.ascontiguousarray(vq_0, dtype=np.float32)
    vqb = np.zeros((P, P), dtype=np.float32)
    vqb[:, :C] = np.tile(vq_0, (P, 1))
    eye = np.eye(P, dtype=np.float32)
    wpos = np.concatenate([eye, eye], axis=1).astype(fp8)
    wneg = np.concatenate([-2 * eye, -2 * eye], axis=1).astype(fp8)
    con = np.zeros((P, 1024), dtype=np.uint8)
    con[:, 0:512] = vqb.view(np.uint8)
    con[:, 512:768] = wpos.view(np.uint8)
    con[:, 768:1024] = wneg.view(np.uint8)
    con = np.ascontiguousarray(con)

    in_maps = []
    for i in range(N_CORES):
        in_maps.append(
            {
                "pred": x[i],
                "m_host": m_host[i],
                "consts_host": con,
            }
        )
    return in_maps


# host-side vq correction: d1 lanes accumulate sum(x) - 2*sum(min) whose
# mask-weighted sum exceeds sum(wm*|d|) by vq_c * S_wm per slab (3 slabs
# per c-group)
D1_CS = [c for cp in range(CP) if LANES[cp] == "d1" for c in (2 * cp, 2 * cp + 1)]


def combine(results, vq_0):
    vq64 = np.asarray(vq_0, dtype=np.float64).reshape(-1)
    vq_d1 = float(vq64[D1_CS].sum())
    num = 0.0
    wsum = 0.0
    for r in results:
        o = np.asarray(r["out"], dtype=np.float64)  # [128, 8] per-partition partials
        s_m = o[:, 2].sum()                  # sum of mask over [128,1024]
        s_wm = float(P * FD) - s_m           # sum of (1-mask), T-replicated
        acc_sum = o[:, 3].sum() + o[:, 4].sum()
        acc_m = o[:, 0].sum() + o[:, 1].sum()
        num += (acc_sum - acc_m) - 3.0 * s_wm * vq_d1
        wsum += s_wm
    den = wsum * float(V * C)  # wsum already counts each mask element T times
    return np.array(num / den, dtype=np.float32)


def kernel(pred, mask_extreme, vq_0):
    nc = _get_nc()
    in_maps = make_in_maps(pred, mask_extreme, vq_0)
    res = run_bass_kernel_spmd(nc, in_maps, core_ids=list(range(N_CORES)))
    return combine(res.results, vq_0)


if __name__ == "__main__":
    rng = np.random.default_rng(0)
    pred = rng.standard_normal((8, V, C, T, H, W), dtype=np.float32)
    mask = rng.integers(0, 2, size=(8, H, W)).astype(np.int32)
    vq = rng.standard_normal((1, C), dtype=np.float32)
    got = kernel(pred=pred, mask_extreme=mask, vq_0=vq)
    m = mask.astype(np.float64)[:, None, None, None, :, :]
    w = 1.0 - m
    p64 = pred.astype(np.float64)
    numr = np.abs(p64 - vq.astype(np.float64)[0][None, None, :, None, None, None]) * w
    exp = numr.sum() / (w.sum() * V * C * T)
    print("kernel:", got, "expected:", exp, "rel:", abs(got - exp) / abs(exp))
